# revision 21
# baseline (speedup 1.0000x reference)
"""Trainium2 Bass kernel for GraphTransitionModel (GNN message passing).

Model (per batch element b, N=256 nodes):
  x[i]   = (obs[b,i], i/N)                              node features, 2-dim
  h1     = relu(W0a^T x_i + W0b^T x_j + a*w4 + b0)      messenger layer 1, 64
  h2     = relu(W1^T h1 + b1)                           64
  h3     = relu(W2^T h2 + b2)                           64
  m(i,j) = w3 . h3 + b3                                 scalar
  msg[i] = sum_j m(i,j) = w3 . (sum_j h3) + N*b3
  u      = MLP_updater([x_i, msg[i]])  (3->64->64->64->1)
  out[b,i] = u

Strategy: pure data parallel, 4 batch elements per core x 8 cores.
Features on partitions, pairs on the free dim; two i-rows (i and
i+128) stacked into 128 partitions so the 64x64 layers run as 128x128
block-diagonal matmuls.

v2: all pair-loop matmuls in bf16 (fp32 matmul = 4 cycles/row on PE),
t-loop processed in chunks of CH=4 so h2 is evacuated as one chunked
ACT op per 4 t's; h3 relu+j-sum split DVE-majority/ACT; h1 built on
DVE in bf16 (2x mode).  PSUM: psA/psB pools 2 bufs x 2 banks each.
"""

import os
import sys
import numpy as np

sys.path.insert(0, "/opt/trn_rl_repo")

B, N, MID = 32, 256, 64
NCORES = 8
BPC = B // NCORES  # batches per core = 4
HALF = N // 2  # 128 stacked tiles per batch
CH = 4  # t's per chunk
NCHUNK = HALF // CH

# wpack column layout (fp32 elements)
C_W1BD = 0
C_W2BD = 128
C_UW1 = 256
C_UW2 = 320
C_W0A = 384
C_W0B = 448
C_UW0 = 512
C_W3S = 576
C_B1S = 578
C_B2S = 579
C_UW3 = 580
C_UB0 = 581
C_UB1 = 582
C_UB2 = 583
C_SCAL = 584
C_TOT = 586


def _build_bass():
    import concourse.bass as bass
    import concourse.bacc as bacc
    import concourse.tile as tile
    from concourse import mybir

    f32 = mybir.dt.float32
    bf16 = mybir.dt.bfloat16
    AF = mybir.ActivationFunctionType
    ALU = mybir.AluOpType

    nc = bacc.Bacc("TRN2", target_bir_lowering=False, num_devices=NCORES)

    wpack_d = nc.declare_dram_parameter("wpack", [128, C_TOT], f32, isOutput=False)
    xT_d = nc.declare_dram_parameter("xT", [BPC, 2, N], f32, isOutput=False)
    ab0_d = nc.declare_dram_parameter("ab0", [BPC, MID, 1], f32, isOutput=False)
    out_d = nc.declare_dram_parameter("out", [BPC, N], f32, isOutput=True)

    with tile.TileContext(nc) as tc:
        with (
            tc.tile_pool(name="consts", bufs=1) as consts,
            tc.tile_pool(name="perb", bufs=2) as perb,
            tc.tile_pool(name="h1p", bufs=2) as h1p,
            tc.tile_pool(name="h2p", bufs=2) as h2p,
            tc.tile_pool(name="work", bufs=8) as work,
            tc.tile_pool(name="ps_a", bufs=2, space="PSUM") as ps_a,
            tc.tile_pool(name="ps_b", bufs=3, space="PSUM") as ps_b,
            tc.tile_pool(name="ps_s", bufs=1, space="PSUM") as ps_s,
        ):
            wp = consts.tile([128, C_TOT], f32, tag="wpack")
            nc.sync.dma_start(out=wp[:], in_=wpack_d[:])
            w1bd = wp[:, C_W1BD : C_W1BD + 128]
            w2bd = wp[:, C_W2BD : C_W2BD + 128]
            uw1 = wp[0:MID, C_UW1 : C_UW1 + MID]
            uw2 = wp[0:MID, C_UW2 : C_UW2 + MID]
            w0a = wp[0:2, C_W0A : C_W0A + MID]
            w0b = wp[0:2, C_W0B : C_W0B + MID]
            uw0 = wp[0:3, C_UW0 : C_UW0 + MID]
            w3s = wp[:, C_W3S : C_W3S + 2]
            b1s = wp[:, C_B1S : C_B1S + 1]
            b2s = wp[:, C_B2S : C_B2S + 1]
            uw3 = wp[0:MID, C_UW3 : C_UW3 + 1]
            ub0 = wp[0:MID, C_UB0 : C_UB0 + 1]
            ub1 = wp[0:MID, C_UB1 : C_UB1 + 1]
            ub2 = wp[0:MID, C_UB2 : C_UB2 + 1]
            scal = wp[:, C_SCAL : C_SCAL + 2]

            # Dummy PE matmul absorbs the wpack-DMA wait so later matmuls
            # (single sync-wait slot) only wait on their RAW producer engine.
            psw = ps_s.tile([1, 1], f32, tag="pss")
            nc.tensor.matmul(psw[:], w1bd[:, 0:1], w1bd[:, 0:1], start=True, stop=True)

            # bf16 copies of the pair-loop weights.  w1b on DVE so the L2
            # matmul's deps are DVE-only; w2b on ACT so L3's are ACT-only.
            w1b = consts.tile([128, 128], bf16, tag="w1b")
            nc.vector.tensor_copy(w1b[:], w1bd)
            w2b = consts.tile([128, 128], bf16, tag="w2b")
            nc.scalar.copy(w2b[:], w2bd)

            zeros = consts.tile([128, N], f32, tag="zeros")
            nc.vector.memset(zeros[:], 0.0)

            # HAM warm-up: ~5us of dense matmuls so the PE clock-gate opens
            # (4/8 -> 8/8) before the pipeline starts; steady-state gaps are
            # far below the ~3.4us re-throttle window so it stays warm.
            wrm = consts.tile([128, N], bf16, tag="wrm")
            nc.vector.memset(wrm[:], 0.0)
            for _ in range(24):
                pswm = ps_s.tile([128, N], f32, tag="pss")
                nc.tensor.matmul(pswm[:], w1b[:], wrm[:], start=True, stop=True)

            for b in range(BPC):
                # ---- per-batch setup ----
                uin = perb.tile([3, N], f32, tag="uin")
                nc.sync.dma_start(out=uin[0:2, :], in_=xT_d[b])
                ab0s = perb.tile([128, 1], f32, tag="ab0s")
                src = ab0_d[b]
                ab0_bcast = bass.AP(
                    tensor=src.tensor,
                    offset=src.offset,
                    ap=[[0, 2]] + list(src.ap),
                )
                nc.sync.dma_start(out=ab0s[:], in_=ab0_bcast)

                psP = ps_s.tile([MID, N], f32, tag="pss")
                nc.tensor.matmul(psP[:], w0a, uin[0:2, :], start=True, stop=True)
                p1 = perb.tile([MID, N], f32, tag="p1")
                nc.scalar.copy(p1[:], psP[:])

                psQ = ps_s.tile([MID, N], f32, tag="pss")
                nc.tensor.matmul(psQ[:], w0b, uin[0:2, :], start=True, stop=True)
                qb = perb.tile([128, N], f32, tag="qb")
                nc.scalar.activation(qb[0:MID, :], psQ[:], AF.Identity, bias=ab0s[0:MID, :])
                nc.sync.dma_start(out=qb[MID:128, :], in_=qb[0:MID, :])

                pb = perb.tile([128, HALF], f32, tag="pb")
                nc.sync.dma_start(out=pb[0:MID, :], in_=p1[:, 0:HALF])
                nc.sync.dma_start(out=pb[MID:128, :], in_=p1[:, HALF:N])

                # DVE fences: single-producer (DVE) tiles for the pair loop
                qb2 = perb.tile([128, N], bf16, tag="qb2")
                nc.vector.tensor_copy(qb2[:], qb[:])
                pb2 = perb.tile([128, HALF], f32, tag="pb2")
                nc.vector.tensor_copy(pb2[:], pb[:])

                # S2[c, t] = sum_j h3 for i=t (upper) / i=t+HALF (lower)
                # DVE-accumulated columns in s2, ACT-accumulated in s2a
                s2 = perb.tile([128, HALF], f32, tag="s2")
                s2a = perb.tile([128, HALF // CH], f32, tag="s2a")

                # ---- main pair loop, chunks of CH t's ----
                for c in range(NCHUNK):
                    t0 = c * CH
                    h1 = h1p.tile([128, CH * N], bf16, tag="h1")
                    for k in range(CH):
                        # k=0,1 on DVE (feeds MM over h1[0:512]); k=2,3 on
                        # GpSimd (feeds MM over h1[512:1024]) so each matmul
                        # sees a single producer engine.
                        eng = nc.vector if k < 2 else nc.gpsimd
                        eng.tensor_scalar(
                            h1[:, k * N : (k + 1) * N],
                            qb2[:],
                            pb2[:, t0 + k : t0 + k + 1],
                            0.0,
                            ALU.add,
                            ALU.max,
                        )
                    psA = ps_a.tile([128, CH * N], f32, tag="psA")
                    nc.tensor.matmul(
                        psA[:, 0:512], w1b[:], h1[:, 0:512], start=True, stop=True
                    )
                    nc.tensor.matmul(
                        psA[:, 512:1024], w1b[:], h1[:, 512:1024], start=True, stop=True
                    )
                    h2 = h2p.tile([128, CH * N], bf16, tag="h2")
                    nc.scalar.activation(h2[:], psA[:], AF.Relu, bias=b1s)
                    psB0 = ps_b.tile([128, 512], f32, tag="psB")
                    nc.tensor.matmul(
                        psB0[:], w2b[:], h2[:, 0:512], start=True, stop=True
                    )
                    psB1 = ps_b.tile([128, 512], f32, tag="psB")
                    nc.tensor.matmul(
                        psB1[:], w2b[:], h2[:, 512:1024], start=True, stop=True
                    )
                    for k in range(CH):
                        t = t0 + k
                        pbt = psB0 if k < 2 else psB1
                        sl = pbt[:, (k % 2) * N : (k % 2 + 1) * N]
                        h3 = work.tile([128, N], bf16, tag="h3")
                        if k != 0:
                            nc.vector.scalar_tensor_tensor(
                                h3[:], sl, b2s, zeros[:], ALU.add, ALU.max,
                                accum_out=s2[:, t : t + 1],
                            )
                        else:
                            nc.scalar.activation(
                                h3[:], sl, AF.Relu, bias=b2s,
                                accum_out=s2a[:, c : c + 1],
                            )

                # ---- msg = w3s^T @ S2  -> [2, HALF] ----
                s2f = perb.tile([128, HALF], f32, tag="s2f")
                nc.vector.tensor_copy(s2f[:], s2[:])
                sel = bass.AP(tensor=s2f.tensor, offset=s2f.offset,
                              ap=[s2f.ap[0], [CH, HALF // CH]])
                nc.vector.tensor_copy(sel, s2a[:])
                psm = ps_s.tile([2, HALF], f32, tag="pss")
                nc.tensor.matmul(psm[:], w3s, s2f[:], start=True, stop=True)
                msg2 = perb.tile([2, HALF], f32, tag="msg2")
                nc.scalar.activation(msg2[:], psm[:], AF.Identity, bias=scal[0:2, 0:1])
                # flatten [2, HALF] -> row 2 of uin [1, N]
                nc.sync.dma_start(out=uin[2:3, :], in_=msg2[:])

                # ---- updater MLP ----
                psu1 = ps_s.tile([MID, N], f32, tag="pss")
                nc.tensor.matmul(psu1[:], uw0, uin[:], start=True, stop=True)
                t1 = perb.tile([MID, N], f32, tag="t1")
                nc.scalar.activation(t1[:], psu1[:], AF.Relu, bias=ub0)
                psu2 = ps_s.tile([MID, N], f32, tag="pss")
                nc.tensor.matmul(psu2[:], uw1, t1[:], start=True, stop=True)
                t2 = perb.tile([MID, N], f32, tag="t2")
                nc.scalar.activation(t2[:], psu2[:], AF.Relu, bias=ub1)
                psu3 = ps_s.tile([MID, N], f32, tag="pss")
                nc.tensor.matmul(psu3[:], uw2, t2[:], start=True, stop=True)
                t3 = perb.tile([MID, N], f32, tag="t3")
                nc.scalar.activation(t3[:], psu3[:], AF.Relu, bias=ub2)
                pso = ps_s.tile([1, N], f32, tag="pss")
                nc.tensor.matmul(pso[:], uw3, t3[:], start=True, stop=True)
                orow = perb.tile([1, N], f32, tag="orow")
                nc.scalar.activation(orow[:], pso[:], AF.Identity, bias=scal[0:1, 1:2])
                nc.sync.dma_start(out=out_d[b], in_=orow[:])

    nc.compile()
    return nc


def _host_inputs(inputs):
    g = lambda k: np.asarray(inputs[k], np.float32)
    obs, action = g("obs"), g("action")
    m_w0, m_b0, m_w1, m_b1 = g("m_w0"), g("m_b0"), g("m_w1"), g("m_b1")
    m_w2, m_b2, m_w3, m_b3 = g("m_w2"), g("m_b2"), g("m_w3"), g("m_b3")
    u_w0, u_b0, u_w1, u_b1 = g("u_w0"), g("u_b0"), g("u_w1"), g("u_b1")
    u_w2, u_b2, u_w3, u_b3 = g("u_w2"), g("u_b2"), g("u_w3"), g("u_b3")

    coor = np.arange(N, dtype=np.float32) / N
    xT = np.stack([obs, np.broadcast_to(coor, obs.shape)], axis=1)  # [B, 2, N]
    ab0 = (action[:, None] * m_w0[4] + m_b0).astype(np.float32)[..., None]

    wpack = np.zeros((128, C_TOT), np.float32)
    wpack[:MID, C_W1BD : C_W1BD + MID] = m_w1
    wpack[MID:, C_W1BD + MID : C_W1BD + 128] = m_w1
    wpack[:MID, C_W2BD : C_W2BD + MID] = m_w2
    wpack[MID:, C_W2BD + MID : C_W2BD + 128] = m_w2
    wpack[:MID, C_UW1 : C_UW1 + MID] = u_w1
    wpack[:MID, C_UW2 : C_UW2 + MID] = u_w2
    wpack[0:2, C_W0A : C_W0A + MID] = m_w0[0:2]
    wpack[0:2, C_W0B : C_W0B + MID] = m_w0[2:4]
    wpack[0:3, C_UW0 : C_UW0 + MID] = u_w0
    wpack[:MID, C_W3S] = m_w3[:, 0]
    wpack[MID:, C_W3S + 1] = m_w3[:, 0]
    wpack[:MID, C_B1S] = m_b1
    wpack[MID:, C_B1S] = m_b1
    wpack[:MID, C_B2S] = m_b2
    wpack[MID:, C_B2S] = m_b2
    wpack[:MID, C_UW3] = u_w3[:, 0]
    wpack[:MID, C_UB0] = u_b0
    wpack[:MID, C_UB1] = u_b1
    wpack[:MID, C_UB2] = u_b2
    wpack[0:2, C_SCAL] = N * float(m_b3[0])
    wpack[0:2, C_SCAL + 1] = float(u_b3[0])

    in_maps = []
    for c in range(NCORES):
        sl = slice(c * BPC, (c + 1) * BPC)
        in_maps.append(
            dict(
                wpack=wpack,
                xT=np.ascontiguousarray(xT[sl]),
                ab0=np.ascontiguousarray(ab0[sl]),
            )
        )
    return in_maps


def kernel(**inputs) -> np.ndarray:
    in_maps = _host_inputs(inputs)

    from concourse.bass_utils import run_bass_kernel_spmd

    nc = _build_bass()
    res = run_bass_kernel_spmd(
        nc, in_maps, core_ids=list(range(NCORES)),
        trace=bool(int(os.environ.get("KERNEL_TRACE", "0"))),
    )
    out = np.concatenate([r["out"] for r in res.results], axis=0)  # [B, N]
    if res.exec_time_ns is not None:
        print(f"HW exec time: {res.exec_time_ns} ns")
        print(f"mean exec time: {res.mean_exec_time_ns} ns")
    return out.astype(np.float32)


if __name__ == "__main__":
    nc = _build_bass()
    print("bass build OK")


# revision 26
# speedup vs baseline: 2.1358x; 2.1358x over previous
"""Trainium2 Bass kernel for GraphTransitionModel (GNN message passing).

Model (per batch element b, N=256 nodes):
  x[i]   = (obs[b,i], i/N)                              node features, 2-dim
  h1     = relu(W0a^T x_i + W0b^T x_j + a*w4 + b0)      messenger layer 1, 64
  h2     = relu(W1^T h1 + b1)                           64
  h3     = relu(W2^T h2 + b2)                           64
  m(i,j) = w3 . h3 + b3                                 scalar
  msg[i] = sum_j m(i,j) = w3 . (sum_j h3) + N*b3
  u      = MLP_updater([x_i, msg[i]])  (3->64->64->64->1)
  out[b,i] = u

Strategy: pure data parallel, 4 batch elements per core x 8 cores.
Features on partitions, pairs on the free dim; two i-rows (i and
i+128) stacked into 128 partitions so the 64x64 layers run as 128x128
block-diagonal matmuls.

v2: all pair-loop matmuls in bf16 (fp32 matmul = 4 cycles/row on PE),
t-loop processed in chunks of CH=4 so h2 is evacuated as one chunked
ACT op per 4 t's; h3 relu+j-sum split DVE-majority/ACT; h1 built on
DVE in bf16 (2x mode).  PSUM: psA/psB pools 2 bufs x 2 banks each.
"""

import os
import sys
import numpy as np

sys.path.insert(0, "/opt/trn_rl_repo")

B, N, MID = 32, 256, 64
NCORES = 8
BPC = B // NCORES  # batches per core = 4
HALF = N // 2  # 128 stacked tiles per batch
CH = 4  # t's per chunk
NCHUNK = HALF // CH

# wpack column layout (fp32 elements)
C_W1BD = 0
C_W2BD = 128
C_UW1 = 256
C_UW2 = 320
C_W0A = 384
C_W0B = 448
C_UW0 = 512
C_W3S = 576
C_B1S = 578
C_B2S = 579
C_UW3 = 580
C_UB0 = 581
C_UB1 = 582
C_UB2 = 583
C_SCAL = 584
C_TOT = 586


def _build_bass():
    import concourse.bass as bass
    import concourse.bacc as bacc
    import concourse.tile as tile
    from concourse import mybir

    f32 = mybir.dt.float32
    bf16 = mybir.dt.bfloat16
    AF = mybir.ActivationFunctionType
    ALU = mybir.AluOpType

    nc = bacc.Bacc("TRN2", target_bir_lowering=False, num_devices=NCORES)

    wpack_d = nc.declare_dram_parameter("wpack", [128, C_TOT], f32, isOutput=False)
    xT_d = nc.declare_dram_parameter("xT", [BPC, 2, N], f32, isOutput=False)
    ab0_d = nc.declare_dram_parameter("ab0", [BPC, MID, 1], f32, isOutput=False)
    out_d = nc.declare_dram_parameter("out", [BPC, N], f32, isOutput=True)

    with tile.TileContext(nc) as tc:
        with (
            tc.tile_pool(name="consts", bufs=1) as consts,
            tc.tile_pool(name="perb", bufs=2) as perb,
            tc.tile_pool(name="h1p", bufs=2) as h1p,
            tc.tile_pool(name="h2p", bufs=2) as h2p,
            tc.tile_pool(name="work", bufs=8) as work,
            tc.tile_pool(name="ps_a", bufs=2, space="PSUM") as ps_a,
            tc.tile_pool(name="ps_b", bufs=3, space="PSUM") as ps_b,
            tc.tile_pool(name="ps_s", bufs=1, space="PSUM") as ps_s,
        ):
            wp = consts.tile([128, C_TOT], f32, tag="wpack")
            nc.sync.dma_start(out=wp[:], in_=wpack_d[:])
            w1bd = wp[:, C_W1BD : C_W1BD + 128]
            w2bd = wp[:, C_W2BD : C_W2BD + 128]
            uw1 = wp[0:MID, C_UW1 : C_UW1 + MID]
            uw2 = wp[0:MID, C_UW2 : C_UW2 + MID]
            w0a = wp[0:2, C_W0A : C_W0A + MID]
            w0b = wp[0:2, C_W0B : C_W0B + MID]
            uw0 = wp[0:3, C_UW0 : C_UW0 + MID]
            w3s = wp[:, C_W3S : C_W3S + 2]
            b1s = wp[:, C_B1S : C_B1S + 1]
            b2s = wp[:, C_B2S : C_B2S + 1]
            uw3 = wp[0:MID, C_UW3 : C_UW3 + 1]
            ub0 = wp[0:MID, C_UB0 : C_UB0 + 1]
            ub1 = wp[0:MID, C_UB1 : C_UB1 + 1]
            ub2 = wp[0:MID, C_UB2 : C_UB2 + 1]
            scal = wp[:, C_SCAL : C_SCAL + 2]

            # Dummy PE matmul absorbs the wpack-DMA wait so later matmuls
            # (single sync-wait slot) only wait on their RAW producer engine.
            psw = ps_s.tile([1, 1], f32, tag="pss")
            nc.tensor.matmul(psw[:], w1bd[:, 0:1], w1bd[:, 0:1], start=True, stop=True)

            # bf16 copies of the pair-loop weights.  w1b on DVE so the L2
            # matmul's deps are DVE-only; w2b on ACT so L3's are ACT-only.
            w1b = consts.tile([128, 128], bf16, tag="w1b")
            nc.vector.tensor_copy(w1b[:], w1bd)
            w2b = consts.tile([128, 128], bf16, tag="w2b")
            nc.scalar.copy(w2b[:], w2bd)

            zeros = consts.tile([128, N], f32, tag="zeros")
            nc.vector.memset(zeros[:], 0.0)

            # HAM warm-up: ~5us of dense matmuls so the PE clock-gate opens
            # (4/8 -> 8/8) before the pipeline starts; steady-state gaps are
            # far below the ~3.4us re-throttle window so it stays warm.
            wrm = consts.tile([128, N], bf16, tag="wrm")
            nc.vector.memset(wrm[:], 0.0)
            for _ in range(24):
                pswm = ps_s.tile([128, N], f32, tag="pss")
                nc.tensor.matmul(pswm[:], w1b[:], wrm[:], start=True, stop=True)

            for b in range(BPC):
                # ---- per-batch setup ----
                uin = perb.tile([3, N], f32, tag="uin")
                nc.sync.dma_start(out=uin[0:2, :], in_=xT_d[b])
                ab0s = perb.tile([128, 1], f32, tag="ab0s")
                src = ab0_d[b]
                ab0_bcast = bass.AP(
                    tensor=src.tensor,
                    offset=src.offset,
                    ap=[[0, 2]] + list(src.ap),
                )
                nc.sync.dma_start(out=ab0s[:], in_=ab0_bcast)

                psP = ps_s.tile([MID, N], f32, tag="pss")
                nc.tensor.matmul(psP[:], w0a, uin[0:2, :], start=True, stop=True)
                p1 = perb.tile([MID, N], f32, tag="p1")
                nc.scalar.copy(p1[:], psP[:])

                psQ = ps_s.tile([MID, N], f32, tag="pss")
                nc.tensor.matmul(psQ[:], w0b, uin[0:2, :], start=True, stop=True)
                qb = perb.tile([128, N], f32, tag="qb")
                nc.scalar.activation(qb[0:MID, :], psQ[:], AF.Identity, bias=ab0s[0:MID, :])
                nc.sync.dma_start(out=qb[MID:128, :], in_=qb[0:MID, :])

                pb = perb.tile([128, HALF], f32, tag="pb")
                nc.sync.dma_start(out=pb[0:MID, :], in_=p1[:, 0:HALF])
                nc.sync.dma_start(out=pb[MID:128, :], in_=p1[:, HALF:N])

                # DVE fences: single-producer (DVE) tiles for the pair loop
                qb2 = perb.tile([128, N], bf16, tag="qb2")
                nc.vector.tensor_copy(qb2[:], qb[:])
                pb2 = perb.tile([128, HALF], f32, tag="pb2")
                nc.vector.tensor_copy(pb2[:], pb[:])

                # S2[c, t] = sum_j h3 for i=t (upper) / i=t+HALF (lower)
                # DVE-accumulated columns in s2, ACT-accumulated in s2a
                s2 = perb.tile([128, HALF], f32, tag="s2")
                s2a = perb.tile([128, HALF], f32, tag="s2a")

                # ---- main pair loop, chunks of CH t's ----
                for c in range(NCHUNK):
                    t0 = c * CH
                    h1 = h1p.tile([128, CH * N], bf16, tag="h1")
                    for k in range(CH):
                        nc.vector.tensor_scalar(
                            h1[:, k * N : (k + 1) * N],
                            qb2[:],
                            pb2[:, t0 + k : t0 + k + 1],
                            0.0,
                            ALU.add,
                            ALU.max,
                        )
                    psA = ps_a.tile([128, CH * N], f32, tag="psA")
                    nc.tensor.matmul(
                        psA[:, 0:512], w1b[:], h1[:, 0:512], start=True, stop=True
                    )
                    nc.tensor.matmul(
                        psA[:, 512:1024], w1b[:], h1[:, 512:1024], start=True, stop=True
                    )
                    h2 = h2p.tile([128, CH * N], bf16, tag="h2")
                    nc.scalar.activation(h2[:], psA[:], AF.Relu, bias=b1s)
                    psB0 = ps_b.tile([128, 512], f32, tag="psB")
                    nc.tensor.matmul(
                        psB0[:], w2b[:], h2[:, 0:512], start=True, stop=True
                    )
                    psB1 = ps_b.tile([128, 512], f32, tag="psB")
                    nc.tensor.matmul(
                        psB1[:], w2b[:], h2[:, 512:1024], start=True, stop=True
                    )
                    for k in range(CH):
                        t = t0 + k
                        pbt = psB0 if k < 2 else psB1
                        sl = pbt[:, (k % 2) * N : (k % 2 + 1) * N]
                        h3 = work.tile([128, N], bf16, tag="h3")
                        # ACT handles k==0 every chunk and k==2 on odd chunks
                        # (1.5 of 4); DVE the rest (2.5 of 4) — balances
                        # DVE(4xh1+2.5xstt) vs ACT(h2-evac+1.5x(relu+accum)).
                        if not (k == 0 or (k == 2 and c % 2 == 1)):
                            nc.vector.scalar_tensor_tensor(
                                h3[:], sl, b2s, zeros[:], ALU.add, ALU.max,
                                accum_out=s2[:, t : t + 1],
                            )
                        else:
                            nc.scalar.activation(
                                h3[:], sl, AF.Relu, bias=b2s,
                                accum_out=s2a[:, t : t + 1],
                            )

                # ---- msg = w3s^T @ S2  -> [2, HALF] ----
                s2f = perb.tile([128, HALF], f32, tag="s2f")
                nc.vector.tensor_copy(s2f[:], s2[:])
                # fold ACT-accumulated columns (t=0 mod 4; t=6 mod 8) into s2f
                sel1 = bass.AP(tensor=s2f.tensor, offset=s2f.offset,
                               ap=[s2f.ap[0], [CH, HALF // CH]])
                src1 = bass.AP(tensor=s2a.tensor, offset=s2a.offset,
                               ap=[s2a.ap[0], [CH, HALF // CH]])
                nc.vector.tensor_copy(sel1, src1)
                sel2 = bass.AP(tensor=s2f.tensor, offset=s2f.offset + 6,
                               ap=[s2f.ap[0], [8, HALF // 8]])
                src2 = bass.AP(tensor=s2a.tensor, offset=s2a.offset + 6,
                               ap=[s2a.ap[0], [8, HALF // 8]])
                nc.vector.tensor_copy(sel2, src2)
                psm = ps_s.tile([2, HALF], f32, tag="pss")
                nc.tensor.matmul(psm[:], w3s, s2f[:], start=True, stop=True)
                msg2 = perb.tile([2, HALF], f32, tag="msg2")
                nc.scalar.activation(msg2[:], psm[:], AF.Identity, bias=scal[0:2, 0:1])
                # flatten [2, HALF] -> row 2 of uin [1, N]
                nc.sync.dma_start(out=uin[2:3, :], in_=msg2[:])

                # ---- updater MLP ----
                psu1 = ps_s.tile([MID, N], f32, tag="pss")
                nc.tensor.matmul(psu1[:], uw0, uin[:], start=True, stop=True)
                t1 = perb.tile([MID, N], f32, tag="t1")
                nc.scalar.activation(t1[:], psu1[:], AF.Relu, bias=ub0)
                psu2 = ps_s.tile([MID, N], f32, tag="pss")
                nc.tensor.matmul(psu2[:], uw1, t1[:], start=True, stop=True)
                t2 = perb.tile([MID, N], f32, tag="t2")
                nc.scalar.activation(t2[:], psu2[:], AF.Relu, bias=ub1)
                psu3 = ps_s.tile([MID, N], f32, tag="pss")
                nc.tensor.matmul(psu3[:], uw2, t2[:], start=True, stop=True)
                t3 = perb.tile([MID, N], f32, tag="t3")
                nc.scalar.activation(t3[:], psu3[:], AF.Relu, bias=ub2)
                pso = ps_s.tile([1, N], f32, tag="pss")
                nc.tensor.matmul(pso[:], uw3, t3[:], start=True, stop=True)
                orow = perb.tile([1, N], f32, tag="orow")
                nc.scalar.activation(orow[:], pso[:], AF.Identity, bias=scal[0:1, 1:2])
                nc.sync.dma_start(out=out_d[b], in_=orow[:])

    nc.compile()
    return nc


def _host_inputs(inputs):
    g = lambda k: np.asarray(inputs[k], np.float32)
    obs, action = g("obs"), g("action")
    m_w0, m_b0, m_w1, m_b1 = g("m_w0"), g("m_b0"), g("m_w1"), g("m_b1")
    m_w2, m_b2, m_w3, m_b3 = g("m_w2"), g("m_b2"), g("m_w3"), g("m_b3")
    u_w0, u_b0, u_w1, u_b1 = g("u_w0"), g("u_b0"), g("u_w1"), g("u_b1")
    u_w2, u_b2, u_w3, u_b3 = g("u_w2"), g("u_b2"), g("u_w3"), g("u_b3")

    coor = np.arange(N, dtype=np.float32) / N
    xT = np.stack([obs, np.broadcast_to(coor, obs.shape)], axis=1)  # [B, 2, N]
    ab0 = (action[:, None] * m_w0[4] + m_b0).astype(np.float32)[..., None]

    wpack = np.zeros((128, C_TOT), np.float32)
    wpack[:MID, C_W1BD : C_W1BD + MID] = m_w1
    wpack[MID:, C_W1BD + MID : C_W1BD + 128] = m_w1
    wpack[:MID, C_W2BD : C_W2BD + MID] = m_w2
    wpack[MID:, C_W2BD + MID : C_W2BD + 128] = m_w2
    wpack[:MID, C_UW1 : C_UW1 + MID] = u_w1
    wpack[:MID, C_UW2 : C_UW2 + MID] = u_w2
    wpack[0:2, C_W0A : C_W0A + MID] = m_w0[0:2]
    wpack[0:2, C_W0B : C_W0B + MID] = m_w0[2:4]
    wpack[0:3, C_UW0 : C_UW0 + MID] = u_w0
    wpack[:MID, C_W3S] = m_w3[:, 0]
    wpack[MID:, C_W3S + 1] = m_w3[:, 0]
    wpack[:MID, C_B1S] = m_b1
    wpack[MID:, C_B1S] = m_b1
    wpack[:MID, C_B2S] = m_b2
    wpack[MID:, C_B2S] = m_b2
    wpack[:MID, C_UW3] = u_w3[:, 0]
    wpack[:MID, C_UB0] = u_b0
    wpack[:MID, C_UB1] = u_b1
    wpack[:MID, C_UB2] = u_b2
    wpack[0:2, C_SCAL] = N * float(m_b3[0])
    wpack[0:2, C_SCAL + 1] = float(u_b3[0])

    in_maps = []
    for c in range(NCORES):
        sl = slice(c * BPC, (c + 1) * BPC)
        in_maps.append(
            dict(
                wpack=wpack,
                xT=np.ascontiguousarray(xT[sl]),
                ab0=np.ascontiguousarray(ab0[sl]),
            )
        )
    return in_maps


def kernel(**inputs) -> np.ndarray:
    in_maps = _host_inputs(inputs)

    from concourse.bass_utils import run_bass_kernel_spmd

    nc = _build_bass()
    res = run_bass_kernel_spmd(
        nc, in_maps, core_ids=list(range(NCORES)),
        trace=bool(int(os.environ.get("KERNEL_TRACE", "0"))),
    )
    out = np.concatenate([r["out"] for r in res.results], axis=0)  # [B, N]
    if res.exec_time_ns is not None:
        print(f"HW exec time: {res.exec_time_ns} ns")
        print(f"mean exec time: {res.mean_exec_time_ns} ns")
    return out.astype(np.float32)


if __name__ == "__main__":
    nc = _build_bass()
    print("bass build OK")


# revision 29
# speedup vs baseline: 2.6321x; 1.2324x over previous
"""Trainium2 Bass kernel for GraphTransitionModel (GNN message passing).

Model (per batch element b, N=256 nodes):
  x[i]   = (obs[b,i], i/N)                              node features, 2-dim
  h1     = relu(W0a^T x_i + W0b^T x_j + a*w4 + b0)      messenger layer 1, 64
  h2     = relu(W1^T h1 + b1)                           64
  h3     = relu(W2^T h2 + b2)                           64
  m(i,j) = w3 . h3 + b3                                 scalar
  msg[i] = sum_j m(i,j) = w3 . (sum_j h3) + N*b3
  u      = MLP_updater([x_i, msg[i]])  (3->64->64->64->1)
  out[b,i] = u

Strategy: pure data parallel, 4 batch elements per core x 8 cores.
Features on partitions, pairs on the free dim; two i-rows (i and
i+128) stacked into 128 partitions so the 64x64 layers run as 128x128
block-diagonal matmuls.

v2: all pair-loop matmuls in bf16 (fp32 matmul = 4 cycles/row on PE),
t-loop processed in chunks of CH=4 so h2 is evacuated as one chunked
ACT op per 4 t's; h3 relu+j-sum split DVE-majority/ACT; h1 built on
DVE in bf16 (2x mode).  PSUM: psA/psB pools 2 bufs x 2 banks each.
"""

import os
import sys
import numpy as np

sys.path.insert(0, "/opt/trn_rl_repo")

B, N, MID = 32, 256, 64
NCORES = 8
BPC = B // NCORES  # batches per core = 4
HALF = N // 2  # 128 stacked tiles per batch
CH = 4  # t's per chunk
NCHUNK = HALF // CH

# wpack column layout (fp32 elements)
C_W1BD = 0
C_W2BD = 128
C_UW1 = 256
C_UW2 = 320
C_W0A = 384
C_W0B = 448
C_UW0 = 512
C_W3S = 576
C_B1S = 578
C_B2S = 579
C_UW3 = 580
C_UB0 = 581
C_UB1 = 582
C_UB2 = 583
C_SCAL = 584
C_TOT = 586


def _build_bass():
    import concourse.bass as bass
    import concourse.bacc as bacc
    import concourse.tile as tile
    from concourse import mybir

    f32 = mybir.dt.float32
    bf16 = mybir.dt.bfloat16
    AF = mybir.ActivationFunctionType
    ALU = mybir.AluOpType

    nc = bacc.Bacc("TRN2", target_bir_lowering=False, num_devices=NCORES)

    wpack_d = nc.declare_dram_parameter("wpack", [128, C_TOT], f32, isOutput=False)
    xT_d = nc.declare_dram_parameter("xT", [BPC, 2, N], f32, isOutput=False)
    ab0_d = nc.declare_dram_parameter("ab0", [BPC, MID, 1], f32, isOutput=False)
    out_d = nc.declare_dram_parameter("out", [BPC, N], f32, isOutput=True)

    with tile.TileContext(nc) as tc:
        with (
            tc.tile_pool(name="consts", bufs=1) as consts,
            tc.tile_pool(name="perb", bufs=4) as perb,
            tc.tile_pool(name="h1p", bufs=3) as h1p,
            tc.tile_pool(name="h2p", bufs=3) as h2p,
            tc.tile_pool(name="work", bufs=8) as work,
            tc.tile_pool(name="ps_a", bufs=2, space="PSUM") as ps_a,
            tc.tile_pool(name="ps_b", bufs=3, space="PSUM") as ps_b,
            tc.tile_pool(name="ps_s", bufs=1, space="PSUM") as ps_s,
        ):
            wp = consts.tile([128, C_TOT], f32, tag="wpack")
            nc.sync.dma_start(out=wp[:], in_=wpack_d[:])
            w1bd = wp[:, C_W1BD : C_W1BD + 128]
            w2bd = wp[:, C_W2BD : C_W2BD + 128]
            uw1 = wp[0:MID, C_UW1 : C_UW1 + MID]
            uw2 = wp[0:MID, C_UW2 : C_UW2 + MID]
            w0a = wp[0:2, C_W0A : C_W0A + MID]
            w0b = wp[0:2, C_W0B : C_W0B + MID]
            uw0 = wp[0:3, C_UW0 : C_UW0 + MID]
            w3s = wp[:, C_W3S : C_W3S + 2]
            b1s = wp[:, C_B1S : C_B1S + 1]
            b2s = wp[:, C_B2S : C_B2S + 1]
            uw3 = wp[0:MID, C_UW3 : C_UW3 + 1]
            ub0 = wp[0:MID, C_UB0 : C_UB0 + 1]
            ub1 = wp[0:MID, C_UB1 : C_UB1 + 1]
            ub2 = wp[0:MID, C_UB2 : C_UB2 + 1]
            scal = wp[:, C_SCAL : C_SCAL + 2]

            # Dummy PE matmul absorbs the wpack-DMA wait so later matmuls
            # (single sync-wait slot) only wait on their RAW producer engine.
            psw = ps_s.tile([1, 1], f32, tag="pss")
            nc.tensor.matmul(psw[:], w1bd[:, 0:1], w1bd[:, 0:1], start=True, stop=True)

            # bf16 copies of the pair-loop weights.  w1b on DVE so the L2
            # matmul's deps are DVE-only; w2b on ACT so L3's are ACT-only.
            w1b = consts.tile([128, 128], bf16, tag="w1b")
            nc.vector.tensor_copy(w1b[:], w1bd)
            w2b = consts.tile([128, 128], bf16, tag="w2b")
            nc.scalar.copy(w2b[:], w2bd)

            zeros = consts.tile([128, N], f32, tag="zeros")
            nc.vector.memset(zeros[:], 0.0)
            wrm = consts.tile([128, N], bf16, tag="wrm")
            nc.vector.memset(wrm[:], 0.0)

            # ---- per-batch setup, all batches up front ----
            uin_b, qb2_b, pb2_b, s2_b, s2a_b = {}, {}, {}, {}, {}
            for b in range(BPC):
                uin = perb.tile([3, N], f32, tag="uin")
                nc.sync.dma_start(out=uin[0:2, :], in_=xT_d[b])
                ab0s = perb.tile([128, 1], f32, tag="ab0s")
                src = ab0_d[b]
                ab0_bcast = bass.AP(
                    tensor=src.tensor,
                    offset=src.offset,
                    ap=[[0, 2]] + list(src.ap),
                )
                nc.sync.dma_start(out=ab0s[:], in_=ab0_bcast)

                psP = ps_s.tile([MID, N], f32, tag="pss")
                nc.tensor.matmul(psP[:], w0a, uin[0:2, :], start=True, stop=True)
                p1 = perb.tile([MID, N], f32, tag="p1")
                nc.scalar.copy(p1[:], psP[:])

                psQ = ps_s.tile([MID, N], f32, tag="pss")
                nc.tensor.matmul(psQ[:], w0b, uin[0:2, :], start=True, stop=True)
                qb = perb.tile([128, N], f32, tag="qb")
                nc.scalar.activation(qb[0:MID, :], psQ[:], AF.Identity, bias=ab0s[0:MID, :])
                nc.sync.dma_start(out=qb[MID:128, :], in_=qb[0:MID, :])

                pb = perb.tile([128, HALF], f32, tag="pb")
                nc.sync.dma_start(out=pb[0:MID, :], in_=p1[:, 0:HALF])
                nc.sync.dma_start(out=pb[MID:128, :], in_=p1[:, HALF:N])

                # DVE fences: single-producer (DVE) tiles for the pair loop
                qb2 = perb.tile([128, N], bf16, tag="qb2")
                nc.vector.tensor_copy(qb2[:], qb[:])
                pb2 = perb.tile([128, HALF], f32, tag="pb2")
                nc.vector.tensor_copy(pb2[:], pb[:])

                s2 = perb.tile([128, HALF], f32, tag="s2")
                s2a = perb.tile([128, HALF], f32, tag="s2a")
                uin_b[b], qb2_b[b], pb2_b[b] = uin, qb2, pb2
                s2_b[b], s2a_b[b] = s2, s2a

            # HAM warm-up: one PSUM accumulation group (no inter-MM deps, so
            # the matmuls run back-to-back) long enough to open the PE clock
            # gate (4/8 -> 8/8) and bridge until the chunk stream starts.
            # Steady-state PE gaps are far below the ~3.4us re-throttle
            # window, so once warm it stays warm.
            pswm = ps_b.tile([128, 512], f32, tag="psB")
            for i in range(48):
                nc.tensor.matmul(
                    pswm[:, 0:N], w1b[:], wrm[:], start=(i == 0), stop=(i == 47)
                )

            # ---- flattened pair loop over all batches ----
            # Modulo-scheduled emission: round r emits h1/L2 for chunk r,
            # h2/L3 for r-1, h3 for r-2 — so no engine queues a waiting op
            # ahead of ready work (strict-FIFO engine queues).
            TOT = BPC * NCHUNK
            h1_of, psA_of, h2_of, psB_of = {}, {}, {}, {}

            def emit_front(g):
                b, c = divmod(g, NCHUNK)
                qb2, pb2 = qb2_b[b], pb2_b[b]
                t0 = c * CH
                h1 = h1p.tile([128, CH * N], bf16, tag="h1")
                for k in range(CH):
                    nc.vector.tensor_scalar(
                        h1[:, k * N : (k + 1) * N],
                        qb2[:],
                        pb2[:, t0 + k : t0 + k + 1],
                        0.0,
                        ALU.add,
                        ALU.max,
                    )
                psA = ps_a.tile([128, CH * N], f32, tag="psA")
                nc.tensor.matmul(
                    psA[:, 0:512], w1b[:], h1[:, 0:512], start=True, stop=True
                )
                nc.tensor.matmul(
                    psA[:, 512:1024], w1b[:], h1[:, 512:1024], start=True, stop=True
                )
                psA_of[g] = psA

            def emit_mid(g):
                psA = psA_of.pop(g)
                h2 = h2p.tile([128, CH * N], bf16, tag="h2")
                nc.scalar.activation(h2[:], psA[:], AF.Relu, bias=b1s)
                psB0 = ps_b.tile([128, 512], f32, tag="psB")
                nc.tensor.matmul(psB0[:], w2b[:], h2[:, 0:512], start=True, stop=True)
                psB1 = ps_b.tile([128, 512], f32, tag="psB")
                nc.tensor.matmul(psB1[:], w2b[:], h2[:, 512:1024], start=True, stop=True)
                psB_of[g] = (psB0, psB1)

            def emit_back(g):
                b, c = divmod(g, NCHUNK)
                s2, s2a = s2_b[b], s2a_b[b]
                psB0, psB1 = psB_of.pop(g)
                t0 = c * CH
                for k in range(CH):
                    t = t0 + k
                    pbt = psB0 if k < 2 else psB1
                    sl = pbt[:, (k % 2) * N : (k % 2 + 1) * N]
                    h3 = work.tile([128, N], bf16, tag="h3")
                    # ACT handles k==0 every chunk and k==2 on odd chunks
                    # (1.5 of 4); DVE the rest (2.5 of 4) — balances
                    # DVE(4xh1+2.5xstt) vs ACT(h2-evac+1.5x(relu+accum)).
                    if not (k == 0 or (k == 2 and c % 2 == 1)):
                        nc.vector.scalar_tensor_tensor(
                            h3[:], sl, b2s, zeros[:], ALU.add, ALU.max,
                            accum_out=s2[:, t : t + 1],
                        )
                    else:
                        nc.scalar.activation(
                            h3[:], sl, AF.Relu, bias=b2s,
                            accum_out=s2a[:, t : t + 1],
                        )
                if c == NCHUNK - 1:
                    emit_tail(b)

            def emit_tail(b):
                uin, s2, s2a = uin_b[b], s2_b[b], s2a_b[b]
                # ---- msg = w3s^T @ S2  -> [2, HALF] ----
                s2f = perb.tile([128, HALF], f32, tag="s2f")
                nc.vector.tensor_copy(s2f[:], s2[:])
                # fold ACT-accumulated cols (t=0 mod 4; t=6 mod 8) into s2f
                sel1 = bass.AP(tensor=s2f.tensor, offset=s2f.offset,
                               ap=[s2f.ap[0], [CH, HALF // CH]])
                src1 = bass.AP(tensor=s2a.tensor, offset=s2a.offset,
                               ap=[s2a.ap[0], [CH, HALF // CH]])
                nc.vector.tensor_copy(sel1, src1)
                sel2 = bass.AP(tensor=s2f.tensor, offset=s2f.offset + 6,
                               ap=[s2f.ap[0], [8, HALF // 8]])
                src2 = bass.AP(tensor=s2a.tensor, offset=s2a.offset + 6,
                               ap=[s2a.ap[0], [8, HALF // 8]])
                nc.vector.tensor_copy(sel2, src2)
                psm = ps_s.tile([2, HALF], f32, tag="pss")
                nc.tensor.matmul(psm[:], w3s, s2f[:], start=True, stop=True)
                msg2 = perb.tile([2, HALF], f32, tag="msg2")
                nc.scalar.activation(msg2[:], psm[:], AF.Identity, bias=scal[0:2, 0:1])
                nc.sync.dma_start(out=uin[2:3, :], in_=msg2[:])

                # ---- updater MLP ----
                psu1 = ps_s.tile([MID, N], f32, tag="pss")
                nc.tensor.matmul(psu1[:], uw0, uin[:], start=True, stop=True)
                t1 = perb.tile([MID, N], f32, tag="t1")
                nc.scalar.activation(t1[:], psu1[:], AF.Relu, bias=ub0)
                psu2 = ps_s.tile([MID, N], f32, tag="pss")
                nc.tensor.matmul(psu2[:], uw1, t1[:], start=True, stop=True)
                t2 = perb.tile([MID, N], f32, tag="t2")
                nc.scalar.activation(t2[:], psu2[:], AF.Relu, bias=ub1)
                psu3 = ps_s.tile([MID, N], f32, tag="pss")
                nc.tensor.matmul(psu3[:], uw2, t2[:], start=True, stop=True)
                t3 = perb.tile([MID, N], f32, tag="t3")
                nc.scalar.activation(t3[:], psu3[:], AF.Relu, bias=ub2)
                pso = ps_s.tile([1, N], f32, tag="pss")
                nc.tensor.matmul(pso[:], uw3, t3[:], start=True, stop=True)
                orow = perb.tile([1, N], f32, tag="orow")
                nc.scalar.activation(orow[:], pso[:], AF.Identity, bias=scal[0:1, 1:2])
                nc.sync.dma_start(out=out_d[b], in_=orow[:])

            for r in range(TOT + 2):
                if r < TOT:
                    emit_front(r)
                if 1 <= r <= TOT:
                    emit_mid(r - 1)
                if r >= 2:
                    emit_back(r - 2)

    nc.compile()
    return nc


def _host_inputs(inputs):
    g = lambda k: np.asarray(inputs[k], np.float32)
    obs, action = g("obs"), g("action")
    m_w0, m_b0, m_w1, m_b1 = g("m_w0"), g("m_b0"), g("m_w1"), g("m_b1")
    m_w2, m_b2, m_w3, m_b3 = g("m_w2"), g("m_b2"), g("m_w3"), g("m_b3")
    u_w0, u_b0, u_w1, u_b1 = g("u_w0"), g("u_b0"), g("u_w1"), g("u_b1")
    u_w2, u_b2, u_w3, u_b3 = g("u_w2"), g("u_b2"), g("u_w3"), g("u_b3")

    coor = np.arange(N, dtype=np.float32) / N
    xT = np.stack([obs, np.broadcast_to(coor, obs.shape)], axis=1)  # [B, 2, N]
    ab0 = (action[:, None] * m_w0[4] + m_b0).astype(np.float32)[..., None]

    wpack = np.zeros((128, C_TOT), np.float32)
    wpack[:MID, C_W1BD : C_W1BD + MID] = m_w1
    wpack[MID:, C_W1BD + MID : C_W1BD + 128] = m_w1
    wpack[:MID, C_W2BD : C_W2BD + MID] = m_w2
    wpack[MID:, C_W2BD + MID : C_W2BD + 128] = m_w2
    wpack[:MID, C_UW1 : C_UW1 + MID] = u_w1
    wpack[:MID, C_UW2 : C_UW2 + MID] = u_w2
    wpack[0:2, C_W0A : C_W0A + MID] = m_w0[0:2]
    wpack[0:2, C_W0B : C_W0B + MID] = m_w0[2:4]
    wpack[0:3, C_UW0 : C_UW0 + MID] = u_w0
    wpack[:MID, C_W3S] = m_w3[:, 0]
    wpack[MID:, C_W3S + 1] = m_w3[:, 0]
    wpack[:MID, C_B1S] = m_b1
    wpack[MID:, C_B1S] = m_b1
    wpack[:MID, C_B2S] = m_b2
    wpack[MID:, C_B2S] = m_b2
    wpack[:MID, C_UW3] = u_w3[:, 0]
    wpack[:MID, C_UB0] = u_b0
    wpack[:MID, C_UB1] = u_b1
    wpack[:MID, C_UB2] = u_b2
    wpack[0:2, C_SCAL] = N * float(m_b3[0])
    wpack[0:2, C_SCAL + 1] = float(u_b3[0])

    in_maps = []
    for c in range(NCORES):
        sl = slice(c * BPC, (c + 1) * BPC)
        in_maps.append(
            dict(
                wpack=wpack,
                xT=np.ascontiguousarray(xT[sl]),
                ab0=np.ascontiguousarray(ab0[sl]),
            )
        )
    return in_maps


def kernel(**inputs) -> np.ndarray:
    in_maps = _host_inputs(inputs)

    from concourse.bass_utils import run_bass_kernel_spmd

    nc = _build_bass()
    res = run_bass_kernel_spmd(
        nc, in_maps, core_ids=list(range(NCORES)),
        trace=bool(int(os.environ.get("KERNEL_TRACE", "0"))),
    )
    out = np.concatenate([r["out"] for r in res.results], axis=0)  # [B, N]
    if res.exec_time_ns is not None:
        print(f"HW exec time: {res.exec_time_ns} ns")
        print(f"mean exec time: {res.mean_exec_time_ns} ns")
    return out.astype(np.float32)


if __name__ == "__main__":
    nc = _build_bass()
    print("bass build OK")


# revision 33
# speedup vs baseline: 3.6451x; 1.3848x over previous
"""Trainium2 Bass kernel for GraphTransitionModel (GNN message passing).

Model (per batch element b, N=256 nodes):
  x[i]   = (obs[b,i], i/N)                              node features, 2-dim
  h1     = relu(W0a^T x_i + W0b^T x_j + a*w4 + b0)      messenger layer 1, 64
  h2     = relu(W1^T h1 + b1)                           64
  h3     = relu(W2^T h2 + b2)                           64
  m(i,j) = w3 . h3 + b3                                 scalar
  msg[i] = sum_j m(i,j) = w3 . (sum_j h3) + N*b3
  u      = MLP_updater([x_i, msg[i]])  (3->64->64->64->1)
  out[b,i] = u

Strategy: pure data parallel, 4 batch elements per core x 8 cores.
Features on partitions, pairs on the free dim; two i-rows (i and
i+128) stacked into 128 partitions so the 64x64 layers run as 128x128
block-diagonal matmuls.

v2: all pair-loop matmuls in bf16 (fp32 matmul = 4 cycles/row on PE),
t-loop processed in chunks of CH=4 so h2 is evacuated as one chunked
ACT op per 4 t's; h3 relu+j-sum split DVE-majority/ACT; h1 built on
DVE in bf16 (2x mode).  PSUM: psA/psB pools 2 bufs x 2 banks each.
"""

import os
import sys
import numpy as np

sys.path.insert(0, "/opt/trn_rl_repo")

B, N, MID = 32, 256, 64
NCORES = 8
BPC = B // NCORES  # batches per core = 4
HALF = N // 2  # 128 stacked tiles per batch
CH = 4  # t's per chunk
NCHUNK = HALF // CH

# wpack column layout (fp32 elements)
C_W1BD = 0
C_W2BD = 128
C_UW1 = 256
C_UW2 = 320
C_W0A = 384
C_W0B = 448
C_UW0 = 512
C_W3S = 576
C_B1S = 578
C_B2S = 579
C_UW3 = 580
C_UB0 = 581
C_UB1 = 582
C_UB2 = 583
C_SCAL = 584
C_TOT = 586


def _build_bass():
    import concourse.bass as bass
    import concourse.bacc as bacc
    import concourse.tile as tile
    from concourse import mybir

    f32 = mybir.dt.float32
    bf16 = mybir.dt.bfloat16
    AF = mybir.ActivationFunctionType
    ALU = mybir.AluOpType

    nc = bacc.Bacc("TRN2", target_bir_lowering=False, num_devices=NCORES)

    wpack_d = nc.declare_dram_parameter("wpack", [128, C_TOT], f32, isOutput=False)
    xT_d = nc.declare_dram_parameter("xT", [BPC, 2, N], f32, isOutput=False)
    ab0_d = nc.declare_dram_parameter("ab0", [BPC, MID, 1], f32, isOutput=False)
    out_d = nc.declare_dram_parameter("out", [BPC, N], f32, isOutput=True)

    with tile.TileContext(nc) as tc:
        with (
            tc.tile_pool(name="consts", bufs=1) as consts,
            tc.tile_pool(name="perb", bufs=4) as perb,
            tc.tile_pool(name="h1p", bufs=4) as h1p,
            tc.tile_pool(name="h2p", bufs=4) as h2p,
            tc.tile_pool(name="work", bufs=8) as work,
            tc.tile_pool(name="ps_a", bufs=4, space="PSUM") as ps_a,
            tc.tile_pool(name="ps_b", bufs=3, space="PSUM") as ps_b,
            tc.tile_pool(name="ps_s", bufs=1, space="PSUM") as ps_s,
        ):
            wp = consts.tile([128, C_TOT], f32, tag="wpack")
            nc.sync.dma_start(out=wp[:], in_=wpack_d[:])
            w1bd = wp[:, C_W1BD : C_W1BD + 128]
            w2bd = wp[:, C_W2BD : C_W2BD + 128]
            uw1 = wp[0:MID, C_UW1 : C_UW1 + MID]
            uw2 = wp[0:MID, C_UW2 : C_UW2 + MID]
            w0a = wp[0:2, C_W0A : C_W0A + MID]
            w0b = wp[0:2, C_W0B : C_W0B + MID]
            uw0 = wp[0:3, C_UW0 : C_UW0 + MID]
            w3s = wp[:, C_W3S : C_W3S + 2]
            b1s = wp[:, C_B1S : C_B1S + 1]
            b2s = wp[:, C_B2S : C_B2S + 1]
            uw3 = wp[0:MID, C_UW3 : C_UW3 + 1]
            ub0 = wp[0:MID, C_UB0 : C_UB0 + 1]
            ub1 = wp[0:MID, C_UB1 : C_UB1 + 1]
            ub2 = wp[0:MID, C_UB2 : C_UB2 + 1]
            scal = wp[:, C_SCAL : C_SCAL + 2]

            # Dummy PE matmul absorbs the wpack-DMA wait so later matmuls
            # (single sync-wait slot) only wait on their RAW producer engine.
            psw = ps_s.tile([1, 1], f32, tag="pss")
            nc.tensor.matmul(psw[:], w1bd[:, 0:1], w1bd[:, 0:1], start=True, stop=True)

            # bf16 copies of the pair-loop weights.  w1b on DVE so the L2
            # matmul's deps are DVE-only; w2b on ACT so L3's are ACT-only.
            w1b = consts.tile([128, 128], bf16, tag="w1b")
            nc.vector.tensor_copy(w1b[:], w1bd)
            w2b = consts.tile([128, 128], bf16, tag="w2b")
            nc.scalar.copy(w2b[:], w2bd)

            zeros = consts.tile([128, N], f32, tag="zeros")
            nc.vector.memset(zeros[:], 0.0)
            wrm = consts.tile([128, N], bf16, tag="wrm")
            nc.vector.memset(wrm[:], 0.0)

            # ---- per-batch setup, all batches up front ----
            uin_b, qb2_b, pb2_b, s2_b, s2a_b = {}, {}, {}, {}, {}
            for b in range(BPC):
                uin = perb.tile([3, N], f32, tag="uin")
                nc.sync.dma_start(out=uin[0:2, :], in_=xT_d[b])
                ab0s = perb.tile([128, 1], f32, tag="ab0s")
                src = ab0_d[b]
                ab0_bcast = bass.AP(
                    tensor=src.tensor,
                    offset=src.offset,
                    ap=[[0, 2]] + list(src.ap),
                )
                nc.sync.dma_start(out=ab0s[:], in_=ab0_bcast)

                psP = ps_s.tile([MID, N], f32, tag="pss")
                nc.tensor.matmul(psP[:], w0a, uin[0:2, :], start=True, stop=True)
                p1 = perb.tile([MID, N], f32, tag="p1")
                nc.scalar.copy(p1[:], psP[:])

                psQ = ps_s.tile([MID, N], f32, tag="pss")
                nc.tensor.matmul(psQ[:], w0b, uin[0:2, :], start=True, stop=True)
                qb = perb.tile([128, N], f32, tag="qb")
                nc.scalar.activation(qb[0:MID, :], psQ[:], AF.Identity, bias=ab0s[0:MID, :])
                nc.sync.dma_start(out=qb[MID:128, :], in_=qb[0:MID, :])

                pb = perb.tile([128, HALF], f32, tag="pb")
                nc.sync.dma_start(out=pb[0:MID, :], in_=p1[:, 0:HALF])
                nc.sync.dma_start(out=pb[MID:128, :], in_=p1[:, HALF:N])

                # DVE fences: single-producer (DVE) tiles for the pair loop
                qb2 = perb.tile([128, N], bf16, tag="qb2")
                nc.vector.tensor_copy(qb2[:], qb[:])
                pb2 = perb.tile([128, HALF], f32, tag="pb2")
                nc.vector.tensor_copy(pb2[:], pb[:])

                s2 = perb.tile([128, HALF], f32, tag="s2")
                s2a = perb.tile([128, HALF], f32, tag="s2a")
                uin_b[b], qb2_b[b], pb2_b[b] = uin, qb2, pb2
                s2_b[b], s2a_b[b] = s2, s2a

            # HAM warm-up: one PSUM accumulation group (no inter-MM deps, so
            # the matmuls run back-to-back) long enough to open the PE clock
            # gate (4/8 -> 8/8) and bridge until the chunk stream starts.
            # Steady-state PE gaps are far below the ~3.4us re-throttle
            # window, so once warm it stays warm.
            pswm = ps_b.tile([128, 512], f32, tag="psB")
            for i in range(48):
                nc.tensor.matmul(
                    pswm[:, 0:N], w1b[:], wrm[:], start=(i == 0), stop=(i == 47)
                )

            # ---- flattened pair loop over all batches ----
            # Half-chunk (2 t's = 512 cols) modulo-scheduled pipeline:
            # round h emits h1+L2 for half-chunk h, h2+L3 for h-1, h3 for
            # h-2 — no engine queues a waiting op ahead of ready work
            # (strict-FIFO engine queues), and the 4-deep psA ring gives
            # the L2->h2->L2 recycle two full chunks of slack.
            TOT = BPC * NCHUNK
            NH = 2 * TOT
            psA_of, h2_of, psB_of = {}, {}, {}

            def emit_front(h):
                g, half = divmod(h, 2)
                b = g // NCHUNK
                qb2, pb2 = qb2_b[b], pb2_b[b]
                t0 = (g % NCHUNK) * CH + half * 2
                h1 = h1p.tile([128, 512], bf16, tag="h1")
                for k in range(2):
                    nc.vector.tensor_scalar(
                        h1[:, k * N : (k + 1) * N],
                        qb2[:],
                        pb2[:, t0 + k : t0 + k + 1],
                        0.0,
                        ALU.add,
                        ALU.max,
                    )
                psA = ps_a.tile([128, 512], f32, tag="psA")
                nc.tensor.matmul(psA[:], w1b[:], h1[:], start=True, stop=True)
                psA_of[h] = psA

            def emit_mid(h):
                psA = psA_of.pop(h)
                h2 = h2p.tile([128, 512], bf16, tag="h2")
                nc.scalar.activation(h2[:], psA[:], AF.Relu, bias=b1s)
                psB = ps_b.tile([128, 512], f32, tag="psB")
                nc.tensor.matmul(psB[:], w2b[:], h2[:], start=True, stop=True)
                psB_of[h] = psB

            def emit_back(h):
                g, half = divmod(h, 2)
                b, c = divmod(g, NCHUNK)
                s2, s2a = s2_b[b], s2a_b[b]
                psB = psB_of.pop(h)
                t0 = c * CH + half * 2
                for k in range(2):
                    t = t0 + k
                    sl = psB[:, k * N : (k + 1) * N]
                    h3 = work.tile([128, N], bf16, tag="h3")
                    # ACT takes the first t of each even half (t = 0 mod 4,
                    # 1 of 4); DVE the rest — balances DVE(2xh1+1.5xstt)
                    # vs ACT(h2-evac+0.5x(relu+accum)) per half-chunk.
                    if k == 0 and half == 0:
                        nc.scalar.activation(
                            h3[:], sl, AF.Relu, bias=b2s,
                            accum_out=s2a[:, t : t + 1],
                        )
                    else:
                        nc.vector.scalar_tensor_tensor(
                            h3[:], sl, b2s, zeros[:], ALU.add, ALU.max,
                            accum_out=s2[:, t : t + 1],
                        )
                if c == NCHUNK - 1 and half == 1:
                    emit_tail(b)

            def emit_tail(b):
                uin, s2, s2a = uin_b[b], s2_b[b], s2a_b[b]
                # ---- msg = w3s^T @ S2  -> [2, HALF] ----
                s2f = perb.tile([128, HALF], f32, tag="s2f")
                nc.vector.tensor_copy(s2f[:], s2[:])
                # fold ACT-accumulated cols (t = 0 mod 4) into s2f
                sel1 = bass.AP(tensor=s2f.tensor, offset=s2f.offset,
                               ap=[s2f.ap[0], [CH, HALF // CH]])
                src1 = bass.AP(tensor=s2a.tensor, offset=s2a.offset,
                               ap=[s2a.ap[0], [CH, HALF // CH]])
                nc.vector.tensor_copy(sel1, src1)
                psm = ps_s.tile([2, HALF], f32, tag="pss")
                nc.tensor.matmul(psm[:], w3s, s2f[:], start=True, stop=True)
                msg2 = perb.tile([2, HALF], f32, tag="msg2")
                nc.scalar.activation(msg2[:], psm[:], AF.Identity, bias=scal[0:2, 0:1])
                nc.sync.dma_start(out=uin[2:3, :], in_=msg2[:])

                # ---- updater MLP ----
                psu1 = ps_s.tile([MID, N], f32, tag="pss")
                nc.tensor.matmul(psu1[:], uw0, uin[:], start=True, stop=True)
                t1 = perb.tile([MID, N], f32, tag="t1")
                nc.scalar.activation(t1[:], psu1[:], AF.Relu, bias=ub0)
                psu2 = ps_s.tile([MID, N], f32, tag="pss")
                nc.tensor.matmul(psu2[:], uw1, t1[:], start=True, stop=True)
                t2 = perb.tile([MID, N], f32, tag="t2")
                nc.scalar.activation(t2[:], psu2[:], AF.Relu, bias=ub1)
                psu3 = ps_s.tile([MID, N], f32, tag="pss")
                nc.tensor.matmul(psu3[:], uw2, t2[:], start=True, stop=True)
                t3 = perb.tile([MID, N], f32, tag="t3")
                nc.scalar.activation(t3[:], psu3[:], AF.Relu, bias=ub2)
                pso = ps_s.tile([1, N], f32, tag="pss")
                nc.tensor.matmul(pso[:], uw3, t3[:], start=True, stop=True)
                orow = perb.tile([1, N], f32, tag="orow")
                nc.scalar.activation(orow[:], pso[:], AF.Identity, bias=scal[0:1, 1:2])
                nc.sync.dma_start(out=out_d[b], in_=orow[:])

            for h in range(NH + 2):
                if h < NH:
                    emit_front(h)
                if 1 <= h <= NH:
                    emit_mid(h - 1)
                if h >= 2:
                    emit_back(h - 2)

    nc.compile()
    return nc


def _host_inputs(inputs):
    g = lambda k: np.asarray(inputs[k], np.float32)
    obs, action = g("obs"), g("action")
    m_w0, m_b0, m_w1, m_b1 = g("m_w0"), g("m_b0"), g("m_w1"), g("m_b1")
    m_w2, m_b2, m_w3, m_b3 = g("m_w2"), g("m_b2"), g("m_w3"), g("m_b3")
    u_w0, u_b0, u_w1, u_b1 = g("u_w0"), g("u_b0"), g("u_w1"), g("u_b1")
    u_w2, u_b2, u_w3, u_b3 = g("u_w2"), g("u_b2"), g("u_w3"), g("u_b3")

    coor = np.arange(N, dtype=np.float32) / N
    xT = np.stack([obs, np.broadcast_to(coor, obs.shape)], axis=1)  # [B, 2, N]
    ab0 = (action[:, None] * m_w0[4] + m_b0).astype(np.float32)[..., None]

    wpack = np.zeros((128, C_TOT), np.float32)
    wpack[:MID, C_W1BD : C_W1BD + MID] = m_w1
    wpack[MID:, C_W1BD + MID : C_W1BD + 128] = m_w1
    wpack[:MID, C_W2BD : C_W2BD + MID] = m_w2
    wpack[MID:, C_W2BD + MID : C_W2BD + 128] = m_w2
    wpack[:MID, C_UW1 : C_UW1 + MID] = u_w1
    wpack[:MID, C_UW2 : C_UW2 + MID] = u_w2
    wpack[0:2, C_W0A : C_W0A + MID] = m_w0[0:2]
    wpack[0:2, C_W0B : C_W0B + MID] = m_w0[2:4]
    wpack[0:3, C_UW0 : C_UW0 + MID] = u_w0
    wpack[:MID, C_W3S] = m_w3[:, 0]
    wpack[MID:, C_W3S + 1] = m_w3[:, 0]
    wpack[:MID, C_B1S] = m_b1
    wpack[MID:, C_B1S] = m_b1
    wpack[:MID, C_B2S] = m_b2
    wpack[MID:, C_B2S] = m_b2
    wpack[:MID, C_UW3] = u_w3[:, 0]
    wpack[:MID, C_UB0] = u_b0
    wpack[:MID, C_UB1] = u_b1
    wpack[:MID, C_UB2] = u_b2
    wpack[0:2, C_SCAL] = N * float(m_b3[0])
    wpack[0:2, C_SCAL + 1] = float(u_b3[0])

    in_maps = []
    for c in range(NCORES):
        sl = slice(c * BPC, (c + 1) * BPC)
        in_maps.append(
            dict(
                wpack=wpack,
                xT=np.ascontiguousarray(xT[sl]),
                ab0=np.ascontiguousarray(ab0[sl]),
            )
        )
    return in_maps


def kernel(**inputs) -> np.ndarray:
    in_maps = _host_inputs(inputs)

    from concourse.bass_utils import run_bass_kernel_spmd

    nc = _build_bass()
    res = run_bass_kernel_spmd(
        nc, in_maps, core_ids=list(range(NCORES)),
        trace=bool(int(os.environ.get("KERNEL_TRACE", "0"))),
    )
    out = np.concatenate([r["out"] for r in res.results], axis=0)  # [B, N]
    if res.exec_time_ns is not None:
        print(f"HW exec time: {res.exec_time_ns} ns")
        print(f"mean exec time: {res.mean_exec_time_ns} ns")
    return out.astype(np.float32)


if __name__ == "__main__":
    nc = _build_bass()
    print("bass build OK")


# revision 36
# speedup vs baseline: 3.8290x; 1.0505x over previous
"""Trainium2 Bass kernel for GraphTransitionModel (GNN message passing).

Model (per batch element b, N=256 nodes):
  x[i]   = (obs[b,i], i/N)                              node features, 2-dim
  h1     = relu(W0a^T x_i + W0b^T x_j + a*w4 + b0)      messenger layer 1, 64
  h2     = relu(W1^T h1 + b1)                           64
  h3     = relu(W2^T h2 + b2)                           64
  m(i,j) = w3 . h3 + b3                                 scalar
  msg[i] = sum_j m(i,j) = w3 . (sum_j h3) + N*b3
  u      = MLP_updater([x_i, msg[i]])  (3->64->64->64->1)
  out[b,i] = u

Strategy: pure data parallel, 4 batch elements per core x 8 cores.
Features on partitions, pairs on the free dim; two i-rows (i and
i+128) stacked into 128 partitions so the 64x64 layers run as 128x128
block-diagonal matmuls.

v2: all pair-loop matmuls in bf16 (fp32 matmul = 4 cycles/row on PE),
t-loop processed in chunks of CH=4 so h2 is evacuated as one chunked
ACT op per 4 t's; h3 relu+j-sum split DVE-majority/ACT; h1 built on
DVE in bf16 (2x mode).  PSUM: psA/psB pools 2 bufs x 2 banks each.
"""

import os
import sys
import numpy as np

sys.path.insert(0, "/opt/trn_rl_repo")

B, N, MID = 32, 256, 64
NCORES = 8
BPC = B // NCORES  # batches per core = 4
HALF = N // 2  # 128 stacked tiles per batch
CH = 4  # t's per chunk
NCHUNK = HALF // CH

# wpack column layout (fp32 elements)
C_W1BD = 0
C_W2BD = 128
C_UW1 = 256
C_UW2 = 320
C_W0A = 384
C_W0B = 448
C_UW0 = 512
C_W3S = 576
C_B1S = 578
C_B2S = 579
C_UW3 = 580
C_UB0 = 581
C_UB1 = 582
C_UB2 = 583
C_SCAL = 584
C_TOT = 586


def _build_bass():
    import concourse.bass as bass
    import concourse.bacc as bacc
    import concourse.tile as tile
    from concourse import mybir

    f32 = mybir.dt.float32
    bf16 = mybir.dt.bfloat16
    AF = mybir.ActivationFunctionType
    ALU = mybir.AluOpType

    nc = bacc.Bacc("TRN2", target_bir_lowering=False, num_devices=NCORES)

    wpack_d = nc.declare_dram_parameter("wpack", [128, C_TOT], f32, isOutput=False)
    xT_d = nc.declare_dram_parameter("xT", [BPC, 2, N], f32, isOutput=False)
    ab0_d = nc.declare_dram_parameter("ab0", [BPC, MID, 1], f32, isOutput=False)
    out_d = nc.declare_dram_parameter("out", [BPC, N], f32, isOutput=True)

    with tile.TileContext(nc) as tc:
        with (
            tc.tile_pool(name="consts", bufs=1) as consts,
            tc.tile_pool(name="perb", bufs=4) as perb,
            tc.tile_pool(name="h1p", bufs=4) as h1p,
            tc.tile_pool(name="h2p", bufs=4) as h2p,
            tc.tile_pool(name="work", bufs=8) as work,
            tc.tile_pool(name="ps_a", bufs=4, space="PSUM") as ps_a,
            tc.tile_pool(name="ps_b", bufs=3, space="PSUM") as ps_b,
            tc.tile_pool(name="ps_s", bufs=1, space="PSUM") as ps_s,
        ):
            wp = consts.tile([128, C_TOT], f32, tag="wpack")
            nc.sync.dma_start(out=wp[:], in_=wpack_d[:])
            w1bd = wp[:, C_W1BD : C_W1BD + 128]
            w2bd = wp[:, C_W2BD : C_W2BD + 128]
            uw1 = wp[0:MID, C_UW1 : C_UW1 + MID]
            uw2 = wp[0:MID, C_UW2 : C_UW2 + MID]
            w0a = wp[0:2, C_W0A : C_W0A + MID]
            w0b = wp[0:2, C_W0B : C_W0B + MID]
            uw0 = wp[0:3, C_UW0 : C_UW0 + MID]
            w3s = wp[:, C_W3S : C_W3S + 2]
            b1s = wp[:, C_B1S : C_B1S + 1]
            b2s = wp[:, C_B2S : C_B2S + 1]
            uw3 = wp[0:MID, C_UW3 : C_UW3 + 1]
            ub0 = wp[0:MID, C_UB0 : C_UB0 + 1]
            ub1 = wp[0:MID, C_UB1 : C_UB1 + 1]
            ub2 = wp[0:MID, C_UB2 : C_UB2 + 1]
            scal = wp[:, C_SCAL : C_SCAL + 2]

            # Dummy PE matmul absorbs the wpack-DMA wait so later matmuls
            # (single sync-wait slot) only wait on their RAW producer engine.
            psw = ps_s.tile([1, 1], f32, tag="pss")
            nc.tensor.matmul(psw[:], w1bd[:, 0:1], w1bd[:, 0:1], start=True, stop=True)

            # bf16 copies of the pair-loop weights.  w1b on DVE so the L2
            # matmul's deps are DVE-only; w2b on ACT so L3's are ACT-only.
            w1b = consts.tile([128, 128], bf16, tag="w1b")
            nc.vector.tensor_copy(w1b[:], w1bd)
            w2b = consts.tile([128, 128], bf16, tag="w2b")
            nc.scalar.copy(w2b[:], w2bd)

            zeros = consts.tile([128, N], f32, tag="zeros")
            nc.vector.memset(zeros[:], 0.0)
            wrm = consts.tile([128, N], bf16, tag="wrm")
            nc.vector.memset(wrm[:], 0.0)

            # ---- per-batch setup, all batches up front ----
            uin_b, qb2_b, pb2_b, s2_b, s2a_b = {}, {}, {}, {}, {}
            for b in range(BPC):
                uin = perb.tile([3, N], f32, tag="uin")
                nc.sync.dma_start(out=uin[0:2, :], in_=xT_d[b])
                ab0s = perb.tile([128, 1], f32, tag="ab0s")
                src = ab0_d[b]
                ab0_bcast = bass.AP(
                    tensor=src.tensor,
                    offset=src.offset,
                    ap=[[0, 2]] + list(src.ap),
                )
                nc.sync.dma_start(out=ab0s[:], in_=ab0_bcast)

                psP = ps_s.tile([MID, N], f32, tag="pss")
                nc.tensor.matmul(psP[:], w0a, uin[0:2, :], start=True, stop=True)
                p1 = perb.tile([MID, N], f32, tag="p1")
                nc.scalar.copy(p1[:], psP[:])

                psQ = ps_s.tile([MID, N], f32, tag="pss")
                nc.tensor.matmul(psQ[:], w0b, uin[0:2, :], start=True, stop=True)
                qb = perb.tile([128, N], f32, tag="qb")
                nc.scalar.activation(qb[0:MID, :], psQ[:], AF.Identity, bias=ab0s[0:MID, :])
                nc.sync.dma_start(out=qb[MID:128, :], in_=qb[0:MID, :])

                pb = perb.tile([128, HALF], f32, tag="pb")
                nc.sync.dma_start(out=pb[0:MID, :], in_=p1[:, 0:HALF])
                nc.sync.dma_start(out=pb[MID:128, :], in_=p1[:, HALF:N])

                # DVE fences: single-producer (DVE) tiles for the pair loop
                qb2 = perb.tile([128, N], bf16, tag="qb2")
                nc.vector.tensor_copy(qb2[:], qb[:])
                pb2 = perb.tile([128, HALF], f32, tag="pb2")
                nc.vector.tensor_copy(pb2[:], pb[:])

                s2 = perb.tile([128, HALF], f32, tag="s2")
                s2a = perb.tile([128, HALF], f32, tag="s2a")
                uin_b[b], qb2_b[b], pb2_b[b] = uin, qb2, pb2
                s2_b[b], s2a_b[b] = s2, s2a

            # ---- flattened pair loop over all batches ----
            # Half-chunk (2 t's = 512 cols) modulo-scheduled pipeline:
            # round h emits h1+L2 for half-chunk h, h2+L3 for h-1, h3 for
            # h-2 — no engine queues a waiting op ahead of ready work
            # (strict-FIFO engine queues), and the 4-deep psA ring gives
            # the L2->h2->L2 recycle two full chunks of slack.
            TOT = BPC * NCHUNK
            NH = 2 * TOT
            psA_of, h2_of, psB_of = {}, {}, {}

            def emit_front(h):
                g, half = divmod(h, 2)
                b = g // NCHUNK
                qb2, pb2 = qb2_b[b], pb2_b[b]
                t0 = (g % NCHUNK) * CH + half * 2
                h1 = h1p.tile([128, 512], bf16, tag="h1")
                for k in range(2):
                    nc.vector.tensor_scalar(
                        h1[:, k * N : (k + 1) * N],
                        qb2[:],
                        pb2[:, t0 + k : t0 + k + 1],
                        0.0,
                        ALU.add,
                        ALU.max,
                    )
                psA = ps_a.tile([128, 512], f32, tag="psA")
                nc.tensor.matmul(psA[:], w1b[:], h1[:], start=True, stop=True)
                psA_of[h] = psA

            def emit_mid(h):
                psA = psA_of.pop(h)
                h2 = h2p.tile([128, 512], bf16, tag="h2")
                nc.scalar.activation(h2[:], psA[:], AF.Relu, bias=b1s)
                psB = ps_b.tile([128, 512], f32, tag="psB")
                nc.tensor.matmul(psB[:], w2b[:], h2[:], start=True, stop=True)
                psB_of[h] = psB

            def emit_back(h):
                g, half = divmod(h, 2)
                b, c = divmod(g, NCHUNK)
                s2, s2a = s2_b[b], s2a_b[b]
                psB = psB_of.pop(h)
                t0 = c * CH + half * 2
                for k in range(2):
                    t = t0 + k
                    sl = psB[:, k * N : (k + 1) * N]
                    h3 = work.tile([128, N], bf16, tag="h3")
                    # ACT takes t = 0 mod 4 plus t = 2 mod 16 (1.25 of 4
                    # per chunk); DVE the rest (2.75) — balances
                    # DVE(4xh1+2.75xstt) vs ACT(2xh2-evac+1.25x(relu+accum)).
                    if k == 0 and (half == 0 or (half == 1 and c % 4 == 0)):
                        nc.scalar.activation(
                            h3[:], sl, AF.Relu, bias=b2s,
                            accum_out=s2a[:, t : t + 1],
                        )
                    else:
                        nc.vector.scalar_tensor_tensor(
                            h3[:], sl, b2s, zeros[:], ALU.add, ALU.max,
                            accum_out=s2[:, t : t + 1],
                        )
                if c == NCHUNK - 1 and half == 1:
                    emit_tail(b)

            def emit_tail(b):
                uin, s2, s2a = uin_b[b], s2_b[b], s2a_b[b]
                # ---- msg = w3s^T @ S2  -> [2, HALF] ----
                s2f = perb.tile([128, HALF], f32, tag="s2f")
                nc.vector.tensor_copy(s2f[:], s2[:])
                # fold ACT-accumulated cols (t=0 mod 4; t=2 mod 16) into s2f
                sel1 = bass.AP(tensor=s2f.tensor, offset=s2f.offset,
                               ap=[s2f.ap[0], [CH, HALF // CH]])
                src1 = bass.AP(tensor=s2a.tensor, offset=s2a.offset,
                               ap=[s2a.ap[0], [CH, HALF // CH]])
                nc.vector.tensor_copy(sel1, src1)
                sel2 = bass.AP(tensor=s2f.tensor, offset=s2f.offset + 2,
                               ap=[s2f.ap[0], [16, HALF // 16]])
                src2 = bass.AP(tensor=s2a.tensor, offset=s2a.offset + 2,
                               ap=[s2a.ap[0], [16, HALF // 16]])
                nc.vector.tensor_copy(sel2, src2)
                psm = ps_s.tile([2, HALF], f32, tag="pss")
                nc.tensor.matmul(psm[:], w3s, s2f[:], start=True, stop=True)
                msg2 = perb.tile([2, HALF], f32, tag="msg2")
                nc.scalar.activation(msg2[:], psm[:], AF.Identity, bias=scal[0:2, 0:1])
                nc.sync.dma_start(out=uin[2:3, :], in_=msg2[:])

                # ---- updater MLP ----
                psu1 = ps_s.tile([MID, N], f32, tag="pss")
                nc.tensor.matmul(psu1[:], uw0, uin[:], start=True, stop=True)
                t1 = perb.tile([MID, N], f32, tag="t1")
                nc.scalar.activation(t1[:], psu1[:], AF.Relu, bias=ub0)
                psu2 = ps_s.tile([MID, N], f32, tag="pss")
                nc.tensor.matmul(psu2[:], uw1, t1[:], start=True, stop=True)
                t2 = perb.tile([MID, N], f32, tag="t2")
                nc.scalar.activation(t2[:], psu2[:], AF.Relu, bias=ub1)
                psu3 = ps_s.tile([MID, N], f32, tag="pss")
                nc.tensor.matmul(psu3[:], uw2, t2[:], start=True, stop=True)
                t3 = perb.tile([MID, N], f32, tag="t3")
                nc.scalar.activation(t3[:], psu3[:], AF.Relu, bias=ub2)
                pso = ps_s.tile([1, N], f32, tag="pss")
                nc.tensor.matmul(pso[:], uw3, t3[:], start=True, stop=True)
                orow = perb.tile([1, N], f32, tag="orow")
                nc.scalar.activation(orow[:], pso[:], AF.Identity, bias=scal[0:1, 1:2])
                nc.sync.dma_start(out=out_d[b], in_=orow[:])

            for h in range(NH + 2):
                if h < NH:
                    emit_front(h)
                if 1 <= h <= NH:
                    emit_mid(h - 1)
                if h >= 2:
                    emit_back(h - 2)

    nc.compile()
    return nc


def _host_inputs(inputs):
    g = lambda k: np.asarray(inputs[k], np.float32)
    obs, action = g("obs"), g("action")
    m_w0, m_b0, m_w1, m_b1 = g("m_w0"), g("m_b0"), g("m_w1"), g("m_b1")
    m_w2, m_b2, m_w3, m_b3 = g("m_w2"), g("m_b2"), g("m_w3"), g("m_b3")
    u_w0, u_b0, u_w1, u_b1 = g("u_w0"), g("u_b0"), g("u_w1"), g("u_b1")
    u_w2, u_b2, u_w3, u_b3 = g("u_w2"), g("u_b2"), g("u_w3"), g("u_b3")

    coor = np.arange(N, dtype=np.float32) / N
    xT = np.stack([obs, np.broadcast_to(coor, obs.shape)], axis=1)  # [B, 2, N]
    ab0 = (action[:, None] * m_w0[4] + m_b0).astype(np.float32)[..., None]

    wpack = np.zeros((128, C_TOT), np.float32)
    wpack[:MID, C_W1BD : C_W1BD + MID] = m_w1
    wpack[MID:, C_W1BD + MID : C_W1BD + 128] = m_w1
    wpack[:MID, C_W2BD : C_W2BD + MID] = m_w2
    wpack[MID:, C_W2BD + MID : C_W2BD + 128] = m_w2
    wpack[:MID, C_UW1 : C_UW1 + MID] = u_w1
    wpack[:MID, C_UW2 : C_UW2 + MID] = u_w2
    wpack[0:2, C_W0A : C_W0A + MID] = m_w0[0:2]
    wpack[0:2, C_W0B : C_W0B + MID] = m_w0[2:4]
    wpack[0:3, C_UW0 : C_UW0 + MID] = u_w0
    wpack[:MID, C_W3S] = m_w3[:, 0]
    wpack[MID:, C_W3S + 1] = m_w3[:, 0]
    wpack[:MID, C_B1S] = m_b1
    wpack[MID:, C_B1S] = m_b1
    wpack[:MID, C_B2S] = m_b2
    wpack[MID:, C_B2S] = m_b2
    wpack[:MID, C_UW3] = u_w3[:, 0]
    wpack[:MID, C_UB0] = u_b0
    wpack[:MID, C_UB1] = u_b1
    wpack[:MID, C_UB2] = u_b2
    wpack[0:2, C_SCAL] = N * float(m_b3[0])
    wpack[0:2, C_SCAL + 1] = float(u_b3[0])

    in_maps = []
    for c in range(NCORES):
        sl = slice(c * BPC, (c + 1) * BPC)
        in_maps.append(
            dict(
                wpack=wpack,
                xT=np.ascontiguousarray(xT[sl]),
                ab0=np.ascontiguousarray(ab0[sl]),
            )
        )
    return in_maps


def kernel(**inputs) -> np.ndarray:
    in_maps = _host_inputs(inputs)

    from concourse.bass_utils import run_bass_kernel_spmd

    nc = _build_bass()
    res = run_bass_kernel_spmd(
        nc, in_maps, core_ids=list(range(NCORES)),
        trace=bool(int(os.environ.get("KERNEL_TRACE", "0"))),
    )
    out = np.concatenate([r["out"] for r in res.results], axis=0)  # [B, N]
    if res.exec_time_ns is not None:
        print(f"HW exec time: {res.exec_time_ns} ns")
        print(f"mean exec time: {res.mean_exec_time_ns} ns")
    return out.astype(np.float32)


if __name__ == "__main__":
    nc = _build_bass()
    print("bass build OK")
